# revision 46
# baseline (speedup 1.0000x reference)
"""Distributed brute-force kNN retrieval (cosine similarity) on 8 Trainium2 cores.

Strategy:
  - Shard the feature bank along N across 8 cores (62500 rows each).
  - Host pre-tiles each shard into fp8, grouped so every DMA is one fully
    contiguous HBM block (128 partitions x up-to-48KB per partition).
    Groups are small at the start (so the first matmuls/scans begin early)
    and at the end (so the serial tail after the last DMA is minimal).
  - Each core computes raw dot products q @ f_shard.T with fp8 matmuls
    (fp32 PSUM accumulation). A chunk pair (2j, 2j+1) lands in one PSUM
    bank ([128, 512] tile, 500 used): queries x chunk 2j on partitions
    0-63, queries x chunk 2j+1 on partitions 64-127 via PE column tiling
    (tile_position=(0,64)).
  - DVE Max8/MaxIndex run DIRECTLY on PSUM (no PSUM->SBUF copy), one
    500-col scan per pair; the odd 125th chunk is a final half block.
  - Candidate vals/idx accumulate in SBUF and drain to HBM progressively
    (on both HWDGE rings) so the final output DMA is tiny.
  - Host maps candidates to global rows, rescores them exactly in fp32
    (normalized cosine, same math as the reference), reduces to top-k and
    gathers the data segments.

Safety margin: top-8 of every 500-col block when only the global top-5
is needed makes the device pass insensitive to fp8 rounding (dot-noise
sigma ~1.4 vs. rank margins of tens of sigma); the exact host rescore
then removes all remaining matmul error.
"""

import os
import sys

import numpy as np

import concourse.bacc as bacc
import concourse.mybir as mybir
from concourse.tile import TileContext
from concourse.bass_utils import run_bass_kernel_spmd


def _ensure_ntff_hook():
    """run_bass_kernel_spmd(trace) under axon imports antenv.axon_hooks,
    which this container image lacks. Provide the shim (profiling works) or
    disable tracing so a stray BASS_TRACE env var cannot crash the run."""
    try:
        import antenv.axon_hooks  # noqa: F401
        return
    except ImportError:
        pass
    try:
        import types
        from trn_agent_boot.trn_boot import _ntff_profile_via_ctypes
        hook = _ntff_profile_via_ctypes("/opt/axon/libaxon_pjrt.so")
        mod = types.ModuleType("antenv.axon_hooks")
        mod.get_axon_ntff_profile_hook = lambda: hook
        mod.set_axon_ntff_profile_hook = lambda h: None
        sys.modules["antenv.axon_hooks"] = mod
        import antenv
        antenv.axon_hooks = mod
    except Exception:
        os.environ["BASS_NEVER_TRACE"] = "1"

# Problem geometry (hardcoded per spec).
B = 64             # queries
D = 768            # feature dim
N = 500000         # feature rows
NCORES = 8
NSH = N // NCORES  # 62500 rows per core
KC = D // 128      # 6 contraction chunks of 128
CHUNK = 500        # matmul moving free dim; PSUM bank holds 512 fp32
NCHUNKS = NSH // CHUNK   # 125
NPAIRS = 62              # pairs (2j, 2j+1) cover chunks 0..123; chunk 124 alone

# DMA groups (chunk counts); contiguous HBM block per group. Fine-grained
# groups keep the matmul/scan pipeline DMA-paced (no bursty group waits);
# small head groups start compute early; small tail groups shrink the
# serial tail after the last byte lands.
GROUPS = [2, 2] + [4] * 27 + [1] + [4, 4] + [2, 1, 1]
assert sum(GROUPS) == NCHUNKS
# Stream order: pairs 0-55 (chunks 0-111), then the lone chunk 124 (so its
# matmul+scan leave the serial tail), then pairs 56-61 (chunks 112-123);
# the final pair's chunks arrive as 1-chunk groups so its matmul halves
# pipeline with the last bytes.
CHUNK_ORDER = list(range(112)) + [NCHUNKS - 1] + list(range(112, NCHUNKS - 1))
CHUNK_POS = {c: p for p, c in enumerate(CHUNK_ORDER)}
# Block execution order: pairs 0-55, lone block, pairs 56-61 (matches the
# stream; load_until positions stay monotone). Output columns unchanged.
BLOCK_EXEC = list(range(56)) + [NPAIRS] + list(range(56, NPAIRS))
GW = max(GROUPS)
PERCH = KC * CHUNK  # bytes per partition per chunk (fp8) = 3000

NBLOCKS = NPAIRS + 1  # one 500-col scan per pair + the lone chunk 124
TOPB = 8
OUTW = NBLOCKS * TOPB  # 504
# Progressive output drains after these block indices (prefix col ranges).
# Pair 61 executes last; its drain covers pairs 56-61 plus the lone
# block's columns (written mid-run).
DRAINS = [(30, 0, 31 * TOPB), (55, 31 * TOPB, 56 * TOPB), (NPAIRS - 1, 56 * TOPB, OUTW)]

_COMPILED = None
LAST_RESULTS = None  # test harness introspection


def _build():
    nc = bacc.Bacc("TRN2", target_bir_lowering=False, debug=False)
    qT = nc.declare_dram_parameter("qT", [128, KC * B], mybir.dt.float8e4, isOutput=False)
    fT = nc.declare_dram_parameter("fT", [NSH * D], mybir.dt.float8e4, isOutput=False)
    out_vals = nc.declare_dram_parameter(
        "vals", [128, OUTW], mybir.dt.float32, isOutput=True
    )
    out_idx = nc.declare_dram_parameter(
        "idx", [128, OUTW], mybir.dt.uint16, isOutput=True
    )

    with TileContext(nc) as tc:
        with (
            tc.tile_pool(name="qpool", bufs=1) as qpool,
            tc.tile_pool(name="fpool", bufs=16) as fpool,
            tc.tile_pool(name="outpool", bufs=1) as outpool,
            tc.tile_pool(name="psum", bufs=8, space="PSUM") as psump,
        ):
            q_sb = qpool.tile([128, KC, B], mybir.dt.float8e4)
            nc.scalar.dma_start(
                out=q_sb[:], in_=qT.ap().rearrange("p (k m) -> p k m", k=KC)
            )

            vals_st = outpool.tile([128, OUTW], mybir.dt.float32)
            idx_st = outpool.tile([128, OUTW], mybir.dt.uint16)

            chunk_views = {}   # chunk id -> SBUF AP [128, KC, CHUNK]
            loaded = [0]
            goff = [0]         # flat fp8 offset of next group
            gidx = [0]

            def load_until(c):
                pos = CHUNK_POS[c]
                while loaded[0] <= pos:
                    gw = GROUPS[gidx[0]]
                    f_sb = fpool.tile([128, GW * PERCH], mybir.dt.float8e4)
                    sz = gw * PERCH
                    nc.sync.dma_start(
                        out=f_sb[:, :sz],
                        in_=fT.ap()[goff[0] : goff[0] + 128 * sz].rearrange(
                            "(p n) -> p n", p=128
                        ),
                    )
                    for ci in range(gw):
                        chunk_views[CHUNK_ORDER[loaded[0] + ci]] = f_sb[
                            :, ci * PERCH : (ci + 1) * PERCH
                        ].rearrange("p (k n) -> p k n", k=KC)
                    goff[0] += 128 * sz
                    loaded[0] += gw
                    gidx[0] += 1

            def mm_half(ps_cols, chunk, half):
                for k in range(KC):
                    nc.tensor.matmul(
                        ps_cols[half * B : (half + 1) * B, :],
                        lhsT=q_sb[:, k, :],
                        rhs=chunk_views[chunk][:, k, :],
                        start=(k == 0),
                        stop=(k == KC - 1),
                        tile_position=(0, half * B) if half else None,
                    )

            for blk in BLOCK_EXEC:
                ps = psump.tile([128, 512], mybir.dt.float32)
                if blk < NPAIRS:
                    load_until(2 * blk)
                    mm_half(ps[:, :CHUNK], 2 * blk, 0)
                    load_until(2 * blk + 1)
                    mm_half(ps[:, :CHUNK], 2 * blk + 1, 1)
                else:  # lone chunk 124: partitions 64-127 scan stale PSUM,
                    # and the host drops those slots (lone block, h==1).
                    load_until(NCHUNKS - 1)
                    mm_half(ps[:, :CHUNK], NCHUNKS - 1, 0)
                scan = ps[:, :CHUNK]
                vslot = vals_st[:, blk * TOPB : (blk + 1) * TOPB]
                nc.vector.max(out=vslot, in_=scan)
                nc.vector.max_index(
                    out=idx_st[:, blk * TOPB : (blk + 1) * TOPB],
                    in_max=vslot,
                    in_values=scan,
                )
                for dblk, c0, c1 in DRAINS:
                    if blk == dblk:
                        # Mid-stream drains must stay OFF the sync ring: the
                        # HWDGE queue is in-order, so a drain gated on DVE
                        # progress would block the feature groups behind it.
                        # Only the final idx drain (no features left) uses
                        # sync, so the two last drains complete in parallel.
                        nc.scalar.dma_start(
                            out=out_vals.ap()[:, c0:c1], in_=vals_st[:, c0:c1]
                        )
                        idx_ring = nc.sync if blk == NPAIRS - 1 else nc.scalar
                        idx_ring.dma_start(
                            out=out_idx.ap()[:, c0:c1], in_=idx_st[:, c0:c1]
                        )

    nc.compile()
    return nc


def _get_compiled():
    global _COMPILED
    if _COMPILED is None:
        _COMPILED = _build()
    return _COMPILED


def _pretile(f_shard, F8):
    """[62500, 768] fp32 -> flat fp8 buffer in per-group contiguous layout
    following CHUNK_ORDER: group g -> [128 partitions][chunk][KC][500],
    partition-major."""
    f8 = f_shard.astype(F8)
    parts = []
    pos = 0
    for gw in GROUPS:
        ids = CHUNK_ORDER[pos : pos + gw]
        rows = np.concatenate([f8[c * CHUNK : (c + 1) * CHUNK] for c in ids])
        sub = rows.reshape(gw, CHUNK, KC, 128)            # (ci, j, k, p)
        parts.append(np.ascontiguousarray(sub.transpose(3, 0, 2, 1)).reshape(-1))
        pos += gw
    return np.concatenate(parts)


def _candidates(idx_arr, val_arr):
    """Map device outputs (128, 504) to per-query (rows, vals).

    Row q < 64 covers the first chunk of each pair (h=0); row q+64 the
    second (h=1). Block b < 62 is pair b; block 62 is the lone chunk 124
    (valid only for h=0). Returns (B, 2*504); invalid slots get -inf val.
    """
    blk = np.repeat(np.arange(NBLOCKS), TOPB)  # (504,)
    lone = blk == NPAIRS
    rows_out = np.empty((B, 2 * OUTW), dtype=np.int64)
    vals_out = np.empty((B, 2 * OUTW), dtype=np.float64)
    for h in (0, 1):
        i = idx_arr[h * B : (h + 1) * B].astype(np.int64)       # (64, 504)
        v = val_arr[h * B : (h + 1) * B].astype(np.float64)
        feat = np.where(lone, (NCHUNKS - 1) * CHUNK + i, (2 * blk + h) * CHUNK + i)
        if h == 1:  # lone chunk block has no h=1 half
            v = np.where(lone, -np.inf, v)
        rows_out[:, h * OUTW : (h + 1) * OUTW] = feat
        vals_out[:, h * OUTW : (h + 1) * OUTW] = v
    return rows_out, vals_out


def kernel(query_feature, feature, data, k=5, **kwargs):
    global LAST_RESULTS
    q = np.ascontiguousarray(np.asarray(query_feature, dtype=np.float32))
    f = np.asarray(feature, dtype=np.float32)
    data = np.asarray(data)
    k = int(k)
    assert q.shape == (B, D) and f.shape == (N, D)

    nc = _get_compiled()

    F8 = mybir.dt.np(mybir.dt.float8e4)
    # qT[p, k*64+m] = q[m, k*128+p]
    qT = np.ascontiguousarray(
        q.astype(F8).reshape(B, KC, 128).transpose(2, 1, 0)
    ).reshape(128, KC * B)
    in_maps = []
    for i in range(NCORES):
        in_maps.append({"qT": qT, "fT": _pretile(f[i * NSH : (i + 1) * NSH], F8)})

    _ensure_ntff_hook()
    res = run_bass_kernel_spmd(nc, in_maps, core_ids=list(range(NCORES)))
    LAST_RESULTS = res

    all_rows, all_vals = [], []
    for i in range(NCORES):
        rows, vals = _candidates(res.results[i]["idx"], res.results[i]["vals"])
        all_rows.append(i * NSH + rows)
        all_vals.append(vals)
    cand_all = np.concatenate(all_rows, axis=1)  # (B, NCORES*1008)
    vals_all = np.concatenate(all_vals, axis=1)

    # Prefilter by device dot value (fp8 noise sigma ~1.4 on margins ~30
    # sigma): keep the top PREK per query, then rescore those exactly.
    PREK = 96
    pre = np.argpartition(-vals_all, PREK, axis=1)[:, :PREK]
    cand = np.take_along_axis(cand_all, pre, axis=1)  # (B, PREK)

    # Exact fp32 rescore of candidates (same math as the reference).
    qn = q / np.linalg.norm(q, axis=1, keepdims=True)
    fc = f[cand]  # (B, C, D)
    fn = fc / np.linalg.norm(fc, axis=2, keepdims=True)
    sims = np.einsum("bd,bcd->bc", qn, fn)  # fp32

    # Final top-k with jax.lax.top_k tie-breaking (value desc, index asc).
    # Exact fp32 ties inside a block can make Max8/MaxIndex emit duplicate
    # candidates: sort by index, mask duplicate neighbors.
    o = np.argsort(cand, axis=1, kind="stable")
    cand_s = np.take_along_axis(cand, o, axis=1)
    sims_s = np.take_along_axis(sims, o, axis=1)
    dup = np.zeros_like(sims_s, dtype=bool)
    dup[:, 1:] = cand_s[:, 1:] == cand_s[:, :-1]
    sims_s = np.where(dup, -np.inf, sims_s)
    sel = np.argsort(-sims_s, axis=1, kind="stable")[:, :k]
    top_idx = np.take_along_axis(cand_s, sel, axis=1)  # (B, k)

    return data[top_idx]  # (B, k, data_cols), input dtype preserved


# revision 50
# speedup vs baseline: 1.1350x; 1.1350x over previous
"""Distributed brute-force kNN retrieval (cosine similarity) on 8 Trainium2 cores.

Strategy:
  - Shard the feature bank along N across 8 cores (62500 rows each).
  - Host pre-tiles each shard into fp8, grouped so every DMA is one fully
    contiguous HBM block (128 partitions x up-to-48KB per partition).
    Groups are small at the start (so the first matmuls/scans begin early)
    and at the end (so the serial tail after the last DMA is minimal).
  - Each core computes raw dot products q @ f_shard.T with fp8 matmuls
    (fp32 PSUM accumulation). A chunk pair (2j, 2j+1) lands in one PSUM
    bank ([128, 512] tile, 500 used): queries x chunk 2j on partitions
    0-63, queries x chunk 2j+1 on partitions 64-127 via PE column tiling
    (tile_position=(0,64)).
  - DVE Max8/MaxIndex run DIRECTLY on PSUM (no PSUM->SBUF copy), one
    500-col scan per pair; the odd 125th chunk is a final half block.
  - Candidate vals/idx accumulate in SBUF and drain to HBM progressively
    (on both HWDGE rings) so the final output DMA is tiny.
  - Host maps candidates to global rows, rescores them exactly in fp32
    (normalized cosine, same math as the reference), reduces to top-k and
    gathers the data segments.

Safety margin: top-8 of every 500-col block when only the global top-5
is needed makes the device pass insensitive to fp8 rounding (dot-noise
sigma ~1.4 vs. rank margins of tens of sigma); the exact host rescore
then removes all remaining matmul error.
"""

import os
import sys

import numpy as np

import concourse.bacc as bacc
import concourse.mybir as mybir
from concourse.tile import TileContext
from concourse.bass_utils import run_bass_kernel_spmd


def _ensure_ntff_hook():
    """run_bass_kernel_spmd(trace) under axon imports antenv.axon_hooks,
    which this container image lacks. Provide the shim (profiling works) or
    disable tracing so a stray BASS_TRACE env var cannot crash the run."""
    try:
        import antenv.axon_hooks  # noqa: F401
        return
    except ImportError:
        pass
    try:
        import types
        from trn_agent_boot.trn_boot import _ntff_profile_via_ctypes
        hook = _ntff_profile_via_ctypes("/opt/axon/libaxon_pjrt.so")
        mod = types.ModuleType("antenv.axon_hooks")
        mod.get_axon_ntff_profile_hook = lambda: hook
        mod.set_axon_ntff_profile_hook = lambda h: None
        sys.modules["antenv.axon_hooks"] = mod
        import antenv
        antenv.axon_hooks = mod
    except Exception:
        os.environ["BASS_NEVER_TRACE"] = "1"

# Problem geometry (hardcoded per spec).
B = 64             # queries
D = 768            # feature dim
N = 500000         # feature rows
NCORES = 8
NSH = N // NCORES  # 62500 rows per core
KC = D // 128      # 6 contraction chunks of 128
CHUNK = 500        # matmul moving free dim; PSUM bank holds 512 fp32
NCHUNKS = NSH // CHUNK   # 125
NPAIRS = 62              # pairs (2j, 2j+1) cover chunks 0..123; chunk 124 alone

# DMA groups (chunk counts); contiguous HBM block per group. Fine-grained
# groups keep the matmul/scan pipeline DMA-paced (no bursty group waits);
# small head groups start compute early; small tail groups shrink the
# serial tail after the last byte lands.
GROUPS = [2, 2] + [4] * 29 + [2, 2, 1]
assert sum(GROUPS) == NCHUNKS
CHUNK_ORDER = list(range(NCHUNKS))
CHUNK_POS = {c: p for p, c in enumerate(CHUNK_ORDER)}
BLOCK_EXEC = list(range(NPAIRS + 1))
GW = max(GROUPS)
PERCH = KC * CHUNK  # bytes per partition per chunk (fp8) = 3000

NBLOCKS = NPAIRS + 1  # one 500-col scan per pair + the lone chunk 124
TOPB = 8
OUTW = NBLOCKS * TOPB  # 504
# Progressive output drains after these block indices (prefix col ranges).
DRAINS = [(30, 0, 31 * TOPB), (55, 31 * TOPB, 56 * TOPB), (NBLOCKS - 1, 56 * TOPB, OUTW)]

_COMPILED = None
LAST_RESULTS = None  # test harness introspection


def _build():
    nc = bacc.Bacc("TRN2", target_bir_lowering=False, debug=False)
    qT = nc.declare_dram_parameter("qT", [128, KC * B], mybir.dt.float8e4, isOutput=False)
    fT = nc.declare_dram_parameter("fT", [NSH * D], mybir.dt.float8e4, isOutput=False)
    out_vals = nc.declare_dram_parameter(
        "vals", [128, OUTW], mybir.dt.float32, isOutput=True
    )
    out_idx = nc.declare_dram_parameter(
        "idx", [128, OUTW], mybir.dt.uint16, isOutput=True
    )

    with TileContext(nc) as tc:
        with (
            tc.tile_pool(name="qpool", bufs=1) as qpool,
            tc.tile_pool(name="fpool", bufs=17) as fpool,
            tc.tile_pool(name="outpool", bufs=1) as outpool,
            tc.tile_pool(name="psum", bufs=8, space="PSUM") as psump,
        ):
            q_sb = qpool.tile([128, KC, B], mybir.dt.float8e4)
            nc.scalar.dma_start(
                out=q_sb[:], in_=qT.ap().rearrange("p (k m) -> p k m", k=KC)
            )

            vals_st = outpool.tile([128, OUTW], mybir.dt.float32)
            idx_st = outpool.tile([128, OUTW], mybir.dt.uint16)

            chunk_views = {}   # chunk id -> SBUF AP [128, KC, CHUNK]
            loaded = [0]
            goff = [0]         # flat fp8 offset of next group
            gidx = [0]

            def load_until(c):
                pos = CHUNK_POS[c]
                while loaded[0] <= pos:
                    gw = GROUPS[gidx[0]]
                    f_sb = fpool.tile([128, GW * PERCH], mybir.dt.float8e4)
                    sz = gw * PERCH
                    nc.sync.dma_start(
                        out=f_sb[:, :sz],
                        in_=fT.ap()[goff[0] : goff[0] + 128 * sz].rearrange(
                            "(p n) -> p n", p=128
                        ),
                    )
                    for ci in range(gw):
                        chunk_views[CHUNK_ORDER[loaded[0] + ci]] = f_sb[
                            :, ci * PERCH : (ci + 1) * PERCH
                        ].rearrange("p (k n) -> p k n", k=KC)
                    goff[0] += 128 * sz
                    loaded[0] += gw
                    gidx[0] += 1

            def mm_half(ps_cols, chunk, half):
                for k in range(KC):
                    nc.tensor.matmul(
                        ps_cols[half * B : (half + 1) * B, :],
                        lhsT=q_sb[:, k, :],
                        rhs=chunk_views[chunk][:, k, :],
                        start=(k == 0),
                        stop=(k == KC - 1),
                        tile_position=(0, half * B) if half else None,
                    )

            for blk in BLOCK_EXEC:
                ps = psump.tile([128, 512], mybir.dt.float32)
                if blk < NPAIRS:
                    load_until(2 * blk)
                    mm_half(ps[:, :CHUNK], 2 * blk, 0)
                    load_until(2 * blk + 1)
                    mm_half(ps[:, :CHUNK], 2 * blk + 1, 1)
                else:  # lone chunk 124: partitions 64-127 scan stale PSUM,
                    # and the host drops those slots (lone block, h==1).
                    load_until(NCHUNKS - 1)
                    mm_half(ps[:, :CHUNK], NCHUNKS - 1, 0)
                scan = ps[:, :CHUNK]
                vslot = vals_st[:, blk * TOPB : (blk + 1) * TOPB]
                nc.vector.max(out=vslot, in_=scan)
                nc.vector.max_index(
                    out=idx_st[:, blk * TOPB : (blk + 1) * TOPB],
                    in_max=vslot,
                    in_values=scan,
                )
                for dblk, c0, c1 in DRAINS:
                    if blk == dblk:
                        # Mid-stream drains must stay OFF the sync ring: the
                        # HWDGE queue is in-order, so a drain gated on DVE
                        # progress would block the feature groups behind it.
                        # Only the final idx drain (no features left) uses
                        # sync, so the two last drains complete in parallel.
                        nc.scalar.dma_start(
                            out=out_vals.ap()[:, c0:c1], in_=vals_st[:, c0:c1]
                        )
                        idx_ring = nc.sync if blk == NBLOCKS - 1 else nc.scalar
                        idx_ring.dma_start(
                            out=out_idx.ap()[:, c0:c1], in_=idx_st[:, c0:c1]
                        )

    nc.compile()
    return nc


def _get_compiled():
    global _COMPILED
    if _COMPILED is None:
        _COMPILED = _build()
    return _COMPILED


def _pretile(f_shard, F8):
    """[62500, 768] fp32 -> flat fp8 buffer in per-group contiguous layout
    following CHUNK_ORDER: group g -> [128 partitions][chunk][KC][500],
    partition-major."""
    f8 = f_shard.astype(F8)
    parts = []
    pos = 0
    for gw in GROUPS:
        ids = CHUNK_ORDER[pos : pos + gw]
        rows = np.concatenate([f8[c * CHUNK : (c + 1) * CHUNK] for c in ids])
        sub = rows.reshape(gw, CHUNK, KC, 128)            # (ci, j, k, p)
        parts.append(np.ascontiguousarray(sub.transpose(3, 0, 2, 1)).reshape(-1))
        pos += gw
    return np.concatenate(parts)


def _candidates(idx_arr, val_arr):
    """Map device outputs (128, 504) to per-query (rows, vals).

    Row q < 64 covers the first chunk of each pair (h=0); row q+64 the
    second (h=1). Block b < 62 is pair b; block 62 is the lone chunk 124
    (valid only for h=0). Returns (B, 2*504); invalid slots get -inf val.
    """
    blk = np.repeat(np.arange(NBLOCKS), TOPB)  # (504,)
    lone = blk == NPAIRS
    rows_out = np.empty((B, 2 * OUTW), dtype=np.int64)
    vals_out = np.empty((B, 2 * OUTW), dtype=np.float64)
    for h in (0, 1):
        i = idx_arr[h * B : (h + 1) * B].astype(np.int64)       # (64, 504)
        v = val_arr[h * B : (h + 1) * B].astype(np.float64)
        feat = np.where(lone, (NCHUNKS - 1) * CHUNK + i, (2 * blk + h) * CHUNK + i)
        if h == 1:  # lone chunk block has no h=1 half
            v = np.where(lone, -np.inf, v)
        rows_out[:, h * OUTW : (h + 1) * OUTW] = feat
        vals_out[:, h * OUTW : (h + 1) * OUTW] = v
    return rows_out, vals_out


def kernel(query_feature, feature, data, k=5, **kwargs):
    global LAST_RESULTS
    q = np.ascontiguousarray(np.asarray(query_feature, dtype=np.float32))
    f = np.asarray(feature, dtype=np.float32)
    data = np.asarray(data)
    k = int(k)
    assert q.shape == (B, D) and f.shape == (N, D)

    nc = _get_compiled()

    F8 = mybir.dt.np(mybir.dt.float8e4)
    # qT[p, k*64+m] = q[m, k*128+p]
    qT = np.ascontiguousarray(
        q.astype(F8).reshape(B, KC, 128).transpose(2, 1, 0)
    ).reshape(128, KC * B)
    in_maps = []
    for i in range(NCORES):
        in_maps.append({"qT": qT, "fT": _pretile(f[i * NSH : (i + 1) * NSH], F8)})

    _ensure_ntff_hook()
    res = run_bass_kernel_spmd(nc, in_maps, core_ids=list(range(NCORES)))
    LAST_RESULTS = res

    all_rows, all_vals = [], []
    for i in range(NCORES):
        rows, vals = _candidates(res.results[i]["idx"], res.results[i]["vals"])
        all_rows.append(i * NSH + rows)
        all_vals.append(vals)
    cand_all = np.concatenate(all_rows, axis=1)  # (B, NCORES*1008)
    vals_all = np.concatenate(all_vals, axis=1)

    # Prefilter by device dot value (fp8 noise sigma ~1.4 on margins ~30
    # sigma): keep the top PREK per query, then rescore those exactly.
    PREK = 96
    pre = np.argpartition(-vals_all, PREK, axis=1)[:, :PREK]
    cand = np.take_along_axis(cand_all, pre, axis=1)  # (B, PREK)

    # Exact fp32 rescore of candidates (same math as the reference).
    qn = q / np.linalg.norm(q, axis=1, keepdims=True)
    fc = f[cand]  # (B, C, D)
    fn = fc / np.linalg.norm(fc, axis=2, keepdims=True)
    sims = np.einsum("bd,bcd->bc", qn, fn)  # fp32

    # Final top-k with jax.lax.top_k tie-breaking (value desc, index asc).
    # Exact fp32 ties inside a block can make Max8/MaxIndex emit duplicate
    # candidates: sort by index, mask duplicate neighbors.
    o = np.argsort(cand, axis=1, kind="stable")
    cand_s = np.take_along_axis(cand, o, axis=1)
    sims_s = np.take_along_axis(sims, o, axis=1)
    dup = np.zeros_like(sims_s, dtype=bool)
    dup[:, 1:] = cand_s[:, 1:] == cand_s[:, :-1]
    sims_s = np.where(dup, -np.inf, sims_s)
    sel = np.argsort(-sims_s, axis=1, kind="stable")[:, :k]
    top_idx = np.take_along_axis(cand_s, sel, axis=1)  # (B, k)

    return data[top_idx]  # (B, k, data_cols), input dtype preserved
